# revision 34
# baseline (speedup 1.0000x reference)
"""Causal multi-head attention on 8 trn2 NeuronCores.

Problem: B=2, T=2048, C=1024, H=16 heads, D=64, fp32, causal softmax.

Sharding: core = b*4 + g handles batch b and heads [4g, 4g+4).
Data parallel over batch (2) x tensor parallel over heads (4 groups).

Per-core math (all layouts chosen so NO on-device transposes are needed):
  inputs: xT_aug [1025, 2048] = [x[b].T ; ones-row]   (bias folded via aug row)
          wq/wk  [1025, 256]  = [W[:, head-slice] ; bias-row]
          wv     [1025, 260]  = per head 64 V cols + 1 "ones" col (0 weights,
                                bias-row 1) so V_sbuf tiles carry the ones
                                column used to accumulate softmax denominators
          wo     [256, 1024]  = Wo[head-slice, :]
  QT[d, t] = wq.T @ xT_aug          (weights stationary)
  V[t, d]  = xT_aug.T @ wv          (xT tiles stationary)
  S^T[s,t] = KT_h.T-free matmul: lhsT=KT_h[d, s-tile], rhs=QT_h[d, t-chunk]
  P = exp(S^T/8) with causal: strictly-upper (s>t) tiles skipped, diagonal
      128-blocks get an additive -1e30 mask; no max-subtraction (logits ~N(0,1))
  [A^T ; denom] = accumulate over s-tiles of lhsT=[V_tile|ones], rhs=P-tile
  AT_norm = A^T * broadcast(1/denom)   (broadcast via K=1 matmul)
  out_partial[t, c] = lhsT=AT_norm tiles, rhs=wo
Host sums the 4 partials per batch and adds bo.
"""

import sys

sys.path.insert(0, "/opt/trn_rl_repo")

from contextlib import ExitStack

import numpy as np

import concourse.bass as bass
import concourse.bacc as bacc
import concourse.mybir as mybir
import concourse.tile as tile
from concourse.bass_utils import run_bass_kernel_spmd

T = 2048
C = 1024
H = 16
D = 64
HPC = 4  # heads per core
DA = HPC * D  # 256 head dims per core
KA = C + 1  # contraction dim incl. bias row
NK = 9  # k-tiles (8 full + 1 bias row)
P = 128
CHUNK = 512
NCHUNK = T // CHUNK  # 4
NT = T // P  # 16
VW = HPC * (D + 1)  # 260: V cols + ones col per head
F32 = mybir.dt.float32
F32R = mybir.dt.float32r
EXPF = mybir.ActivationFunctionType.Exp


def build_nc():
    nc = bacc.Bacc("TRN2", target_bir_lowering=False)
    xT = nc.dram_tensor("xT", [KA, T], F32R, kind="ExternalInput")
    wq = nc.dram_tensor("wq", [KA, DA], F32R, kind="ExternalInput")
    wk = nc.dram_tensor("wk", [KA, DA], F32R, kind="ExternalInput")
    wv = nc.dram_tensor("wv", [KA, VW], F32R, kind="ExternalInput")
    wo = nc.dram_tensor("wo", [DA, C], F32R, kind="ExternalInput")
    mask = nc.dram_tensor("mask", [P, P], F32, kind="ExternalInput")
    out = nc.dram_tensor("out", [T, C], F32, kind="ExternalOutput")

    with ExitStack() as ctx:
        tc = ctx.enter_context(tile.TileContext(nc))
        persist = ctx.enter_context(tc.tile_pool(name="persist", bufs=1))

        QT = [persist.tile([P, T], F32R, tag=f"qt{i}", name=f"qt{i}") for i in range(2)]
        KT = [persist.tile([P, T], F32R, tag=f"kt{i}", name=f"kt{i}") for i in range(2)]
        Vt = [persist.tile([P, VW], F32R, tag=f"v{i}", name=f"v{i}") for i in range(NT)]
        AT = [persist.tile([P, T], F32R, tag=f"at{i}", name=f"at{i}") for i in range(2)]
        WO = [persist.tile([P, C], F32R, tag=f"wo{i}", name=f"wo{i}") for i in range(2)]
        MSK = persist.tile([P, P], F32, tag="mask")

        # ---- interleaved per-chunk: projections -> attention -> out-proj ----
        # causal structure means chunk j's attention only reads K/V for
        # s <= chunk j end, all produced by proj chunks <= j; interleaving
        # lets ACT's exp stream (the attention-phase bottleneck) start while
        # PE is still doing later chunks' projections.
        with (
            tc.tile_pool(name="wts", bufs=1) as wpool,
            tc.tile_pool(name="xc", bufs=3) as xpool,
            tc.tile_pool(name="ppsum", bufs=2, space="PSUM") as ppsum,
            tc.tile_pool(name="attn", bufs=4) as apool,
            tc.tile_pool(name="outs", bufs=3) as opool,
            tc.tile_pool(name="apsum", bufs=1, space="PSUM") as apsum,
        ):
            WQb = wpool.tile([P, 8, DA], F32R, tag="wqb", name="wqb")
            WKb = wpool.tile([P, 8, DA], F32R, tag="wkb", name="wkb")
            WVb = wpool.tile([P, 8, VW], F32R, tag="wvb", name="wvb")
            WQr = wpool.tile([1, DA], F32R, tag="wqr", name="wqr")
            WKr = wpool.tile([1, DA], F32R, tag="wkr", name="wkr")
            WVr = wpool.tile([1, VW], F32R, tag="wvr", name="wvr")
            nc.gpsimd.dma_start(out=WQr, in_=wq[C : C + 1, :])
            nc.gpsimd.dma_start(out=WKr, in_=wk[C : C + 1, :])
            nc.gpsimd.dma_start(out=WVr, in_=wv[C : C + 1, :])
            nc.gpsimd.dma_start(out=MSK, in_=mask[:, :])
            wqs = wq[0:C, :].rearrange("(k p) c -> p k c", p=P)
            wks = wk[0:C, :].rearrange("(k p) c -> p k c", p=P)
            wvs = wv[0:C, :].rearrange("(k p) c -> p k c", p=P)
            nc.gpsimd.dma_start(out=WQb[:, 0:2, :], in_=wqs[:, 0:2, :])
            nc.gpsimd.dma_start(out=WKb[:, 0:2, :], in_=wks[:, 0:2, :])
            nc.gpsimd.dma_start(out=WQb[:, 2:8, :], in_=wqs[:, 2:8, :])
            nc.gpsimd.dma_start(out=WKb[:, 2:8, :], in_=wks[:, 2:8, :])
            nc.gpsimd.dma_start(out=WVb[:, 0:4, :], in_=wvs[:, 0:4, :])
            nc.gpsimd.dma_start(out=WVb[:, 4:8, :], in_=wvs[:, 4:8, :])
            for i in range(2):
                nc.gpsimd.dma_start(out=WO[i], in_=wo[i * P : (i + 1) * P, :])

            def wq_t(k, dsl):
                return WQb[:, k, dsl] if k < 8 else WQr[:1, dsl]

            def wk_t(k, dsl):
                return WKb[:, k, dsl] if k < 8 else WKr[:1, dsl]

            def wv_t(k):
                return WVb[:, k, :] if k < 8 else WVr[:1, :]

            xtiles = {}

            def emit_x_dma(j):
                """Issue chunk-j x loads (called ~2 chunks ahead)."""
                tsl = slice(j * CHUNK, (j + 1) * CHUNK)
                xcb = xpool.tile([P, 8, CHUNK], F32R, tag="xcb", name="xcb")
                xcr = xpool.tile([1, CHUNK], F32R, tag="xcr", name="xcr")
                xsrc = xT[0:C, tsl].rearrange("(k p) t -> p k t", p=P)
                nq = 4 if j == 0 else 2  # finer first chunk
                w = 8 // nq
                for kq in range(nq):
                    nc.sync.dma_start(
                        out=xcb[:, w * kq : w * kq + w, :],
                        in_=xsrc[:, w * kq : w * kq + w, :],
                    )
                nc.sync.dma_start(out=xcr, in_=xT[C : C + 1, tsl])
                xtiles[j] = (xcb, xcr)

            def emit_proj_pieces(j):
                """Chunk-j projection work as closures."""
                tsl = slice(j * CHUNK, (j + 1) * CHUNK)
                xcb, xcr = xtiles[j]

                def xc_t(k, msl=slice(None)):
                    return xcb[:, k, msl] if k < 8 else xcr[:1, msl]

                def qk(dh, wt, dst):
                    def f():
                        dsl = slice(dh * P, (dh + 1) * P)
                        pp = ppsum.tile([P, CHUNK], F32, tag="pp", name="pp")
                        for k in range(NK):
                            nc.tensor.matmul(
                                pp, wt(k, dsl), xc_t(k),
                                start=(k == 0), stop=(k == NK - 1),
                            )
                        nc.vector.tensor_copy(out=dst[dh][:, tsl], in_=pp)
                    return f

                def vproj(tt):
                    def f():
                        m = j * 4 + tt
                        msl = slice(tt * P, (tt + 1) * P)
                        pv = ppsum.tile([P, CHUNK], F32, tag="pp", name="pv")[
                            :, :VW
                        ]
                        for k in range(NK):
                            nc.tensor.matmul(
                                pv, xc_t(k, msl), wv_t(k),
                                start=(k == 0), stop=(k == NK - 1),
                            )
                        nc.vector.tensor_copy(out=Vt[m], in_=pv)
                    return f

                return [
                    qk(0, wq_t, QT), qk(0, wk_t, KT),
                    vproj(0), vproj(1),
                    qk(1, wq_t, QT), qk(1, wk_t, KT),
                    vproj(2), vproj(3),
                ]

            def emit_outproj_pieces(j):
                pieces = []
                for tt in range(4):
                    for cc in range(2):
                        def f(tt=tt, cc=cc):
                            m = j * 4 + tt
                            msl = slice(m * P, (m + 1) * P)
                            csl = slice(cc * 512, (cc + 1) * 512)
                            ot = opool.tile(
                                [P, 512], F32, tag="ot", name="ot", bufs=4
                            )
                            po = ppsum.tile([P, 512], F32, tag="pp", name="po")
                            for kk in range(2):
                                nc.tensor.matmul(
                                    po,
                                    AT[kk][:, msl],
                                    WO[kk][:, csl],
                                    start=(kk == 0), stop=(kk == 1),
                                )
                            if cc == 0 and j == NCHUNK - 1:
                                nc.scalar.copy(out=ot, in_=po)
                            else:
                                nc.vector.tensor_copy(out=ot, in_=po)
                            if cc == 0:
                                nc.gpsimd.dma_start(out=out[msl, csl], in_=ot)
                            else:
                                nc.sync.dma_start(out=out[msl, csl], in_=ot)
                        pieces.append(f)
                return pieces

            def att_steps(j):
                """Chunk-j attention as an ordered list of closures."""
                tsl = slice(j * CHUNK, (j + 1) * CHUNK)
                n_s = 4 * j + 4
                steps = []
                for hp in range(2):
                    heads = [2 * hp, 2 * hp + 1]
                    hold = {}

                    def start_pair(heads=heads, hold=hold):
                        for hh in heads:
                            hold[hh] = apsum.tile(
                                [D + 1, CHUNK], F32, tag="pA", bufs=3,
                                name=f"pA{hh}",
                            )

                    steps.append(start_pair)
                    for i in range(n_s):
                        for hh in heads:
                            def step(i=i, hh=hh, hold=hold):
                                c0 = (i - 4 * j) * P if i >= 4 * j else 0
                                s0 = i * P
                                dh = hh // 2
                                doff = (hh % 2) * D
                                dsl = slice(doff, doff + D)
                                vsl = slice(
                                    hh * (D + 1), (hh + 1) * (D + 1)
                                )
                                ps = apsum.tile(
                                    [P, CHUNK], F32, tag="ps", bufs=3,
                                    name="ps",
                                )
                                et = apool.tile(
                                    [P, CHUNK], F32R, tag="et", bufs=6,
                                    name="et",
                                )
                                nc.tensor.matmul(
                                    ps[:, c0:],
                                    KT[dh][dsl, s0 : s0 + P],
                                    QT[dh][
                                        dsl,
                                        j * CHUNK + c0 : (j + 1) * CHUNK,
                                    ],
                                    start=True, stop=True,
                                )
                                nc.scalar.activation(
                                    out=et[:, c0:], in_=ps[:, c0:],
                                    func=EXPF, scale=0.125,
                                )
                                if i >= 4 * j:
                                    nc.vector.tensor_mul(
                                        out=et[:, c0 : c0 + P],
                                        in0=et[:, c0 : c0 + P],
                                        in1=MSK,
                                    )
                                nc.tensor.matmul(
                                    hold[hh][:, c0:],
                                    Vt[i][:, vsl],
                                    et[:, c0:],
                                    start=(i == 0), stop=(i == n_s - 1),
                                )

                            steps.append(step)
                    for hh in heads:
                        def norm(hh=hh, hold=hold):
                            dh = hh // 2
                            doff = (hh % 2) * D
                            dsl = slice(doff, doff + D)
                            pA = hold[hh]
                            rc = apool.tile([1, CHUNK], F32, tag="rc", name="rc")
                            nc.vector.reciprocal(out=rc, in_=pA[D : D + 1, :])
                            rb = apool.tile([D, CHUNK], F32, tag="rb", name="rb")
                            nc.gpsimd.partition_broadcast(rb, rc, channels=D)
                            nc.vector.tensor_mul(
                                out=AT[dh][dsl, tsl], in0=pA[:D, :], in1=rb
                            )

                        steps.append(norm)
                return steps

            emit_x_dma(0)
            emit_x_dma(1)
            for piece in emit_proj_pieces(0):
                piece()
            for j in range(NCHUNK):
                if j + 2 < NCHUNK:
                    emit_x_dma(j + 2)
                inter = []
                if j + 1 < NCHUNK:
                    inter += emit_proj_pieces(j + 1)
                if j == NCHUNK - 1:
                    for jj in range(NCHUNK - 1):
                        inter += emit_outproj_pieces(jj)
                steps = att_steps(j)
                m, n = len(steps), len(inter)
                k = 0
                for idx, stepf in enumerate(steps):
                    stepf()
                    while k < n and (idx + 1) * n // m > k:
                        inter[k]()
                        k += 1
            for piece in emit_outproj_pieces(NCHUNK - 1):
                piece()

    nc.finalize()
    return nc


def make_in_maps(x, Wq, bq, Wk, bk, Wv, bv, Wo, bo):
    x = np.asarray(x, np.float32)
    Wq, bq = np.asarray(Wq, np.float32), np.asarray(bq, np.float32)
    Wk, bk = np.asarray(Wk, np.float32), np.asarray(bk, np.float32)
    Wv, bv = np.asarray(Wv, np.float32), np.asarray(bv, np.float32)
    Wo = np.asarray(Wo, np.float32)
    mask = np.where(
        np.arange(P)[:, None] > np.arange(P)[None, :], np.float32(0), np.float32(1)
    ).astype(np.float32)
    in_maps = []
    for core in range(8):
        b, g = divmod(core, 4)
        hs = slice(g * DA, (g + 1) * DA)
        xT_aug = np.ascontiguousarray(
            np.concatenate([x[b].T, np.ones((1, T), np.float32)], 0)
        )
        wq_s = np.concatenate([Wq[:, hs], bq[None, hs]], 0)
        wk_s = np.concatenate([Wk[:, hs], bk[None, hs]], 0)
        wv_s = np.concatenate([Wv[:, hs], bv[None, hs]], 0).reshape(KA, HPC, D)
        ones_col = np.zeros((KA, HPC, 1), np.float32)
        ones_col[C, :, 0] = 1.0
        wv_aug = np.ascontiguousarray(
            np.concatenate([wv_s, ones_col], -1).reshape(KA, VW)
        )
        in_maps.append(
            {
                "xT": xT_aug,
                "wq": np.ascontiguousarray(wq_s),
                "wk": np.ascontiguousarray(wk_s),
                "wv": wv_aug,
                "wo": np.ascontiguousarray(Wo[hs, :]),
                "mask": mask,
            }
        )
    return in_maps


_NC_CACHE = None


def get_nc():
    global _NC_CACHE
    if _NC_CACHE is None:
        _NC_CACHE = build_nc()
    return _NC_CACHE


def kernel(x, Wq, bq, Wk, bk, Wv, bv, Wo, bo, _trace=False):
    nc = get_nc()
    in_maps = make_in_maps(x, Wq, bq, Wk, bk, Wv, bv, Wo, bo)
    res = run_bass_kernel_spmd(nc, in_maps, list(range(8)), trace=_trace)
    kernel.last_results = res
    outs = [res.results[i]["out"] for i in range(8)]
    bo = np.asarray(bo, np.float32)
    full = np.stack(
        [outs[0] + outs[1] + outs[2] + outs[3], outs[4] + outs[5] + outs[6] + outs[7]],
        0,
    ) + bo[None, None, :]
    return full.astype(np.float32)


# revision 35
# speedup vs baseline: 1.0054x; 1.0054x over previous
"""Causal multi-head attention on 8 trn2 NeuronCores.

Problem: B=2, T=2048, C=1024, H=16 heads, D=64, fp32, causal softmax.

Sharding: core = b*4 + g handles batch b and heads [4g, 4g+4).
Data parallel over batch (2) x tensor parallel over heads (4 groups).

Per-core math (all layouts chosen so NO on-device transposes are needed):
  inputs: xT_aug [1025, 2048] = [x[b].T ; ones-row]   (bias folded via aug row)
          wq/wk  [1025, 256]  = [W[:, head-slice] ; bias-row]
          wv     [1025, 260]  = per head 64 V cols + 1 "ones" col (0 weights,
                                bias-row 1) so V_sbuf tiles carry the ones
                                column used to accumulate softmax denominators
          wo     [256, 1024]  = Wo[head-slice, :]
  QT[d, t] = wq.T @ xT_aug          (weights stationary)
  V[t, d]  = xT_aug.T @ wv          (xT tiles stationary)
  S^T[s,t] = KT_h.T-free matmul: lhsT=KT_h[d, s-tile], rhs=QT_h[d, t-chunk]
  P = exp(S^T/8) with causal: strictly-upper (s>t) tiles skipped, diagonal
      128-blocks get an additive -1e30 mask; no max-subtraction (logits ~N(0,1))
  [A^T ; denom] = accumulate over s-tiles of lhsT=[V_tile|ones], rhs=P-tile
  AT_norm = A^T * broadcast(1/denom)   (broadcast via K=1 matmul)
  out_partial[t, c] = lhsT=AT_norm tiles, rhs=wo
Host sums the 4 partials per batch and adds bo.
"""

import sys

sys.path.insert(0, "/opt/trn_rl_repo")

from contextlib import ExitStack

import numpy as np

import concourse.bass as bass
import concourse.bacc as bacc
import concourse.mybir as mybir
import concourse.tile as tile
from concourse.bass_utils import run_bass_kernel_spmd

T = 2048
C = 1024
H = 16
D = 64
HPC = 4  # heads per core
DA = HPC * D  # 256 head dims per core
KA = C + 1  # contraction dim incl. bias row
NK = 9  # k-tiles (8 full + 1 bias row)
P = 128
CHUNK = 512
NCHUNK = T // CHUNK  # 4
NT = T // P  # 16
VW = HPC * (D + 1)  # 260: V cols + ones col per head
F32 = mybir.dt.float32
F32R = mybir.dt.float32r
EXPF = mybir.ActivationFunctionType.Exp


def build_nc():
    nc = bacc.Bacc("TRN2", target_bir_lowering=False)
    xT = nc.dram_tensor("xT", [KA, T], F32R, kind="ExternalInput")
    wq = nc.dram_tensor("wq", [KA, DA], F32R, kind="ExternalInput")
    wk = nc.dram_tensor("wk", [KA, DA], F32R, kind="ExternalInput")
    wv = nc.dram_tensor("wv", [KA, VW], F32R, kind="ExternalInput")
    wo = nc.dram_tensor("wo", [DA, C], F32R, kind="ExternalInput")
    mask = nc.dram_tensor("mask", [P, P], F32, kind="ExternalInput")
    rows = nc.dram_tensor("rows", [1, 2 * DA + VW], F32R, kind="ExternalInput")
    out = nc.dram_tensor("out", [T, C], F32, kind="ExternalOutput")

    with ExitStack() as ctx:
        tc = ctx.enter_context(tile.TileContext(nc))
        persist = ctx.enter_context(tc.tile_pool(name="persist", bufs=1))

        QT = [persist.tile([P, T], F32R, tag=f"qt{i}", name=f"qt{i}") for i in range(2)]
        KT = [persist.tile([P, T], F32R, tag=f"kt{i}", name=f"kt{i}") for i in range(2)]
        Vt = [persist.tile([P, VW], F32R, tag=f"v{i}", name=f"v{i}") for i in range(NT)]
        AT = [persist.tile([P, T], F32R, tag=f"at{i}", name=f"at{i}") for i in range(2)]
        WO = [persist.tile([P, C], F32R, tag=f"wo{i}", name=f"wo{i}") for i in range(2)]
        MSK = persist.tile([P, P], F32, tag="mask")

        # ---- interleaved per-chunk: projections -> attention -> out-proj ----
        # causal structure means chunk j's attention only reads K/V for
        # s <= chunk j end, all produced by proj chunks <= j; interleaving
        # lets ACT's exp stream (the attention-phase bottleneck) start while
        # PE is still doing later chunks' projections.
        with (
            tc.tile_pool(name="wts", bufs=1) as wpool,
            tc.tile_pool(name="xc", bufs=3) as xpool,
            tc.tile_pool(name="ppsum", bufs=2, space="PSUM") as ppsum,
            tc.tile_pool(name="attn", bufs=4) as apool,
            tc.tile_pool(name="outs", bufs=3) as opool,
            tc.tile_pool(name="apsum", bufs=1, space="PSUM") as apsum,
        ):
            WQb = wpool.tile([P, 8, DA], F32R, tag="wqb", name="wqb")
            WKb = wpool.tile([P, 8, DA], F32R, tag="wkb", name="wkb")
            WVb = wpool.tile([P, 8, VW], F32R, tag="wvb", name="wvb")
            ROWS = wpool.tile([1, 2 * DA + VW], F32R, tag="rows", name="rows")
            nc.gpsimd.dma_start(out=ROWS, in_=rows[:, :])
            WQr = ROWS[:, 0:DA]
            WKr = ROWS[:, DA : 2 * DA]
            WVr = ROWS[:, 2 * DA : 2 * DA + VW]
            wqs = wq[0:C, :].rearrange("(k p) c -> p k c", p=P)
            wks = wk[0:C, :].rearrange("(k p) c -> p k c", p=P)
            wvs = wv[0:C, :].rearrange("(k p) c -> p k c", p=P)
            nc.gpsimd.dma_start(out=WQb[:, 0:2, :], in_=wqs[:, 0:2, :])
            nc.gpsimd.dma_start(out=WKb[:, 0:2, :], in_=wks[:, 0:2, :])
            nc.gpsimd.dma_start(out=MSK, in_=mask[:, :])
            nc.gpsimd.dma_start(out=WQb[:, 2:8, :], in_=wqs[:, 2:8, :])
            nc.gpsimd.dma_start(out=WKb[:, 2:8, :], in_=wks[:, 2:8, :])
            nc.gpsimd.dma_start(out=WVb[:, 0:4, :], in_=wvs[:, 0:4, :])
            nc.gpsimd.dma_start(out=WVb[:, 4:8, :], in_=wvs[:, 4:8, :])
            for i in range(2):
                nc.gpsimd.dma_start(out=WO[i], in_=wo[i * P : (i + 1) * P, :])

            def wq_t(k, dsl):
                return WQb[:, k, dsl] if k < 8 else WQr[:1, dsl]

            def wk_t(k, dsl):
                return WKb[:, k, dsl] if k < 8 else WKr[:1, dsl]

            def wv_t(k):
                return WVb[:, k, :] if k < 8 else WVr[:1, :]

            xtiles = {}

            def emit_x_dma(j):
                """Issue chunk-j x loads (called ~2 chunks ahead)."""
                tsl = slice(j * CHUNK, (j + 1) * CHUNK)
                xcb = xpool.tile([P, 8, CHUNK], F32R, tag="xcb", name="xcb")
                xcr = xpool.tile([1, CHUNK], F32R, tag="xcr", name="xcr")
                xsrc = xT[0:C, tsl].rearrange("(k p) t -> p k t", p=P)
                nq = 4 if j == 0 else 2  # finer first chunk
                w = 8 // nq
                for kq in range(nq):
                    nc.sync.dma_start(
                        out=xcb[:, w * kq : w * kq + w, :],
                        in_=xsrc[:, w * kq : w * kq + w, :],
                    )
                nc.sync.dma_start(out=xcr, in_=xT[C : C + 1, tsl])
                xtiles[j] = (xcb, xcr)

            def emit_proj_pieces(j):
                """Chunk-j projection work as closures."""
                tsl = slice(j * CHUNK, (j + 1) * CHUNK)
                xcb, xcr = xtiles[j]

                def xc_t(k, msl=slice(None)):
                    return xcb[:, k, msl] if k < 8 else xcr[:1, msl]

                def qk(dh, wt, dst):
                    def f():
                        dsl = slice(dh * P, (dh + 1) * P)
                        pp = ppsum.tile([P, CHUNK], F32, tag="pp", name="pp")
                        for k in range(NK):
                            nc.tensor.matmul(
                                pp, wt(k, dsl), xc_t(k),
                                start=(k == 0), stop=(k == NK - 1),
                            )
                        nc.vector.tensor_copy(out=dst[dh][:, tsl], in_=pp)
                    return f

                def vproj(tt):
                    def f():
                        m = j * 4 + tt
                        msl = slice(tt * P, (tt + 1) * P)
                        pv = ppsum.tile([P, CHUNK], F32, tag="pp", name="pv")[
                            :, :VW
                        ]
                        for k in range(NK):
                            nc.tensor.matmul(
                                pv, xc_t(k, msl), wv_t(k),
                                start=(k == 0), stop=(k == NK - 1),
                            )
                        nc.vector.tensor_copy(out=Vt[m], in_=pv)
                    return f

                return [
                    qk(0, wq_t, QT), qk(0, wk_t, KT),
                    vproj(0), vproj(1),
                    qk(1, wq_t, QT), qk(1, wk_t, KT),
                    vproj(2), vproj(3),
                ]

            def emit_outproj_pieces(j):
                pieces = []
                for tt in range(4):
                    for cc in range(2):
                        def f(tt=tt, cc=cc):
                            m = j * 4 + tt
                            msl = slice(m * P, (m + 1) * P)
                            csl = slice(cc * 512, (cc + 1) * 512)
                            ot = opool.tile(
                                [P, 512], F32, tag="ot", name="ot", bufs=4
                            )
                            po = ppsum.tile([P, 512], F32, tag="pp", name="po")
                            for kk in range(2):
                                nc.tensor.matmul(
                                    po,
                                    AT[kk][:, msl],
                                    WO[kk][:, csl],
                                    start=(kk == 0), stop=(kk == 1),
                                )
                            if cc == 0 and j == NCHUNK - 1:
                                nc.scalar.copy(out=ot, in_=po)
                            else:
                                nc.vector.tensor_copy(out=ot, in_=po)
                            if cc == 0:
                                nc.gpsimd.dma_start(out=out[msl, csl], in_=ot)
                            else:
                                nc.sync.dma_start(out=out[msl, csl], in_=ot)
                        pieces.append(f)
                return pieces

            def att_steps(j):
                """Chunk-j attention as an ordered list of closures."""
                tsl = slice(j * CHUNK, (j + 1) * CHUNK)
                n_s = 4 * j + 4
                steps = []
                for hp in range(2):
                    heads = [2 * hp, 2 * hp + 1]
                    hold = {}

                    def start_pair(heads=heads, hold=hold):
                        for hh in heads:
                            hold[hh] = apsum.tile(
                                [D + 1, CHUNK], F32, tag="pA", bufs=3,
                                name=f"pA{hh}",
                            )

                    steps.append(start_pair)
                    for i in range(n_s):
                        for hh in heads:
                            def step(i=i, hh=hh, hold=hold):
                                c0 = (i - 4 * j) * P if i >= 4 * j else 0
                                s0 = i * P
                                dh = hh // 2
                                doff = (hh % 2) * D
                                dsl = slice(doff, doff + D)
                                vsl = slice(
                                    hh * (D + 1), (hh + 1) * (D + 1)
                                )
                                ps = apsum.tile(
                                    [P, CHUNK], F32, tag="ps", bufs=3,
                                    name="ps",
                                )
                                et = apool.tile(
                                    [P, CHUNK], F32R, tag="et", bufs=6,
                                    name="et",
                                )
                                nc.tensor.matmul(
                                    ps[:, c0:],
                                    KT[dh][dsl, s0 : s0 + P],
                                    QT[dh][
                                        dsl,
                                        j * CHUNK + c0 : (j + 1) * CHUNK,
                                    ],
                                    start=True, stop=True,
                                )
                                nc.scalar.activation(
                                    out=et[:, c0:], in_=ps[:, c0:],
                                    func=EXPF, scale=0.125,
                                )
                                if i >= 4 * j:
                                    nc.vector.tensor_mul(
                                        out=et[:, c0 : c0 + P],
                                        in0=et[:, c0 : c0 + P],
                                        in1=MSK,
                                    )
                                nc.tensor.matmul(
                                    hold[hh][:, c0:],
                                    Vt[i][:, vsl],
                                    et[:, c0:],
                                    start=(i == 0), stop=(i == n_s - 1),
                                )

                            steps.append(step)
                    for hh in heads:
                        def norm(hh=hh, hold=hold):
                            dh = hh // 2
                            doff = (hh % 2) * D
                            dsl = slice(doff, doff + D)
                            pA = hold[hh]
                            rc = apool.tile([1, CHUNK], F32, tag="rc", name="rc")
                            nc.vector.reciprocal(out=rc, in_=pA[D : D + 1, :])
                            rb = apool.tile([D, CHUNK], F32, tag="rb", name="rb")
                            nc.gpsimd.partition_broadcast(rb, rc, channels=D)
                            nc.vector.tensor_mul(
                                out=AT[dh][dsl, tsl], in0=pA[:D, :], in1=rb
                            )

                        steps.append(norm)
                return steps

            emit_x_dma(0)
            emit_x_dma(1)
            for piece in emit_proj_pieces(0):
                piece()
            for j in range(NCHUNK):
                if j + 2 < NCHUNK:
                    emit_x_dma(j + 2)
                inter = []
                if j + 1 < NCHUNK:
                    inter += emit_proj_pieces(j + 1)
                if j == NCHUNK - 1:
                    for jj in range(NCHUNK - 1):
                        inter += emit_outproj_pieces(jj)
                steps = att_steps(j)
                m, n = len(steps), len(inter)
                k = 0
                for idx, stepf in enumerate(steps):
                    stepf()
                    while k < n and (idx + 1) * n // m > k:
                        inter[k]()
                        k += 1
            for piece in emit_outproj_pieces(NCHUNK - 1):
                piece()

    nc.finalize()
    return nc


def make_in_maps(x, Wq, bq, Wk, bk, Wv, bv, Wo, bo):
    x = np.asarray(x, np.float32)
    Wq, bq = np.asarray(Wq, np.float32), np.asarray(bq, np.float32)
    Wk, bk = np.asarray(Wk, np.float32), np.asarray(bk, np.float32)
    Wv, bv = np.asarray(Wv, np.float32), np.asarray(bv, np.float32)
    Wo = np.asarray(Wo, np.float32)
    mask = np.where(
        np.arange(P)[:, None] > np.arange(P)[None, :], np.float32(0), np.float32(1)
    ).astype(np.float32)
    in_maps = []
    for core in range(8):
        b, g = divmod(core, 4)
        hs = slice(g * DA, (g + 1) * DA)
        xT_aug = np.ascontiguousarray(
            np.concatenate([x[b].T, np.ones((1, T), np.float32)], 0)
        )
        wq_s = np.concatenate([Wq[:, hs], bq[None, hs]], 0)
        wk_s = np.concatenate([Wk[:, hs], bk[None, hs]], 0)
        wv_s = np.concatenate([Wv[:, hs], bv[None, hs]], 0).reshape(KA, HPC, D)
        ones_col = np.zeros((KA, HPC, 1), np.float32)
        ones_col[C, :, 0] = 1.0
        wv_aug = np.ascontiguousarray(
            np.concatenate([wv_s, ones_col], -1).reshape(KA, VW)
        )
        rows_packed = np.concatenate(
            [wq_s[C], wk_s[C], wv_aug[C]]
        ).reshape(1, 2 * DA + VW)
        in_maps.append(
            {
                "xT": xT_aug,
                "rows": np.ascontiguousarray(rows_packed),
                "wq": np.ascontiguousarray(wq_s),
                "wk": np.ascontiguousarray(wk_s),
                "wv": wv_aug,
                "wo": np.ascontiguousarray(Wo[hs, :]),
                "mask": mask,
            }
        )
    return in_maps


_NC_CACHE = None


def get_nc():
    global _NC_CACHE
    if _NC_CACHE is None:
        _NC_CACHE = build_nc()
    return _NC_CACHE


def kernel(x, Wq, bq, Wk, bk, Wv, bv, Wo, bo, _trace=False):
    nc = get_nc()
    in_maps = make_in_maps(x, Wq, bq, Wk, bk, Wv, bv, Wo, bo)
    res = run_bass_kernel_spmd(nc, in_maps, list(range(8)), trace=_trace)
    kernel.last_results = res
    outs = [res.results[i]["out"] for i in range(8)]
    bo = np.asarray(bo, np.float32)
    full = np.stack(
        [outs[0] + outs[1] + outs[2] + outs[3], outs[4] + outs[5] + outs[6] + outs[7]],
        0,
    ) + bo[None, None, :]
    return full.astype(np.float32)
